# revision 20
# baseline (speedup 1.0000x reference)
"""Trainium2 Bass kernel for nn_ASC_LSTM (per-step LSTM encoder/decoder).

Strategy: data-parallel over batch (32 rows/core x 8 cores), weights
replicated and streamed from HBM in bf16 (host-cast). Gates are computed
transposed ([gate_rows, batch] in PSUM) so the recurrent hidden state
stays in [128, 4, 32] K-chunk layout and never needs an on-chip
transpose. Per-step biases are applied by the ScalarE activation that
reads PSUM (sigmoid/tanh with per-partition bias operand).
"""
import os
import sys

import numpy as np
import ml_dtypes

sys.path.insert(0, "/opt/trn_rl_repo")

import concourse.bass as bass
import concourse.tile as tile
from concourse import bacc, mybir
from concourse import bass_utils
from concourse.bass import ts

B, I, H, S, RES = 256, 256, 512, 64, 4
NCORES = 8
BLOC = B // NCORES  # 32
BF16 = mybir.dt.bfloat16
F32 = mybir.dt.float32
AF = mybir.ActivationFunctionType

_STATE = {}


def _build_module():
    nc = bacc.Bacc(
        "TRN2",
        target_bir_lowering=False,
        debug=False,
        enable_asserts=False,
        num_devices=NCORES,
    )
    wt_d = nc.dram_tensor("wt", [S, 128, 6, 16 * 128], BF16, kind="ExternalInput").ap()
    wdt_d = nc.dram_tensor("wdt", [S, 128, 4, 6 * 128], BF16, kind="ExternalInput").ap()
    x_d = nc.dram_tensor("xr", [128, S, 2, BLOC], BF16, kind="ExternalInput").ap()
    benc_d = nc.dram_tensor("benc", [128, S, 16], F32, kind="ExternalInput").ap()
    bdec_d = nc.dram_tensor("bdec", [128, S, 6], F32, kind="ExternalInput").ap()
    out_d = nc.dram_tensor("out", [S, 128, 2, BLOC], F32, kind="ExternalOutput").ap()

    with tile.TileContext(nc) as tc:
        with (
            tc.tile_pool(name="wenc", bufs=4) as wpool,
            tc.tile_pool(name="wdec", bufs=4) as wdpool,
            tc.tile_pool(name="big", bufs=1) as bigpool,
            tc.tile_pool(name="gates", bufs=3) as gpool,
            tc.tile_pool(name="small", bufs=3) as spool,
            tc.tile_pool(name="psum", bufs=8, space="PSUM") as psum,
        ):
            x_sb = bigpool.tile([128, S, 2, BLOC], BF16, tag="xsb")
            nc.sync.dma_start(out=x_sb, in_=x_d)
            benc_sb = bigpool.tile([128, S, 16], F32, tag="benc")
            nc.sync.dma_start(out=benc_sb, in_=benc_d)
            bdec_sb = bigpool.tile([128, S, 6], F32, tag="bdec")
            nc.sync.dma_start(out=bdec_sb, in_=bdec_d)

            enc_hist = bigpool.tile([128, S, 4, BLOC], F32, tag="ehist")
            enc_bf = bigpool.tile([128, S, 4, BLOC], BF16, tag="ebf")

            # ---------------- encoder scan ----------------
            h_bf_prev = None
            for t in range(S):
                w_sb = wpool.tile([128, 6, 16 * 128], BF16, tag="wenc")
                nks = 2 if t == 0 else 6
                # k=0..3 (x + first h chunks): 8 fine-grained HWDGE transfers;
                # k=4,5: 2 large SWDGE transfers (GpSimd issue cost ~1us each)
                for k in range(min(nks, 4)):
                    for hh in range(2):
                        nc.sync.dma_start(
                            out=w_sb[:, k, ts(hh, 8 * 128)],
                            in_=wt_d[t, :, k, ts(hh, 8 * 128)],
                        )
                for k in range(4, nks):
                    nc.gpsimd.dma_start(out=w_sb[:, k], in_=wt_d[t, :, k])
                sig_i = gpool.tile([128, 4, BLOC], F32, tag="sig_i")
                sig_f = gpool.tile([128, 4, BLOC], F32, tag="sig_f")
                tanh_g = gpool.tile([128, 4, BLOC], F32, tag="tanh_g")
                sig_o = gpool.tile([128, 4, BLOC], F32, tag="sig_o")
                for m in range(16):
                    ps = psum.tile([128, BLOC], F32, tag="ps")
                    for k in range(nks):
                        rhs = x_sb[:, t, k] if k < 2 else h_bf_prev[:, k - 2]
                        nc.tensor.matmul(
                            ps,
                            lhsT=w_sb[:, k, ts(m, 128)],
                            rhs=rhs,
                            start=(k == 0),
                            stop=(k == nks - 1),
                        )
                    # bias-add on DVE (per-partition scalar); activations are
                    # applied batched per gate tile below on ScalarE
                    dst = (
                        sig_i[:, m]
                        if m < 4
                        else sig_f[:, m - 4]
                        if m < 8
                        else tanh_g[:, m - 8]
                        if m < 12
                        else sig_o[:, m - 12]
                    )
                    nc.vector.tensor_scalar_add(dst, ps, benc_sb[:, t, m : m + 1])
                nc.scalar.activation(out=sig_i, in_=sig_i, func=AF.Sigmoid)
                nc.scalar.activation(out=sig_f, in_=sig_f, func=AF.Sigmoid)
                nc.scalar.activation(out=tanh_g, in_=tanh_g, func=AF.Tanh)
                nc.scalar.activation(out=sig_o, in_=sig_o, func=AF.Sigmoid)
                # c = sig_f * h_prev + sig_i * tanh_g ; h = sig_o * tanh(c)
                nc.vector.tensor_mul(sig_i, sig_i, tanh_g)
                if t > 0:
                    nc.vector.tensor_mul(sig_f, sig_f, enc_hist[:, t - 1])
                    nc.vector.tensor_add(sig_i, sig_i, sig_f)
                nc.scalar.activation(out=sig_i, in_=sig_i, func=AF.Tanh)
                nc.vector.tensor_mul(enc_hist[:, t], sig_i, sig_o)
                h_bf_prev = gpool.tile([128, 4, BLOC], BF16, tag="hbf")
                nc.vector.tensor_copy(out=h_bf_prev, in_=enc_hist[:, t])
                # fused elu -> bf16 history for the decoder (keeps the raw h
                # in enc_hist for the next-step recurrence)
                etmp = spool.tile([128, 4, BLOC], F32, tag="elut")
                hpos = spool.tile([128, 4, BLOC], F32, tag="hpos")
                nc.vector.tensor_scalar_min(etmp, enc_hist[:, t], 0.0)
                nc.scalar.activation(out=etmp, in_=etmp, func=AF.Exp)
                nc.vector.tensor_scalar_max(hpos, enc_hist[:, t], 0.0)
                nc.vector.tensor_add(hpos, hpos, etmp)
                nc.vector.tensor_scalar_add(enc_bf[:, t], hpos, -1.0)

            # ---------------- skip blend (bf16) ----------------
            for k in range(0, S, RES):
                nc.vector.tensor_add(
                    enc_bf[:, k], enc_bf[:, k], enc_bf[:, (k - RES) % S]
                )
                nc.vector.tensor_scalar_mul(enc_bf[:, k], enc_bf[:, k], 0.5)

            # ---------------- decoder (parallel over idx) ----------------
            run = None
            for idx in range(S):
                tsrc = S - 1 - idx
                wd_sb = wdpool.tile([128, 4, 6 * 128], BF16, tag="wdec")
                for k in range(3):
                    for hh in range(2):
                        nc.sync.dma_start(
                            out=wd_sb[:, k, ts(hh, 3 * 128)],
                            in_=wdt_d[idx, :, k, ts(hh, 3 * 128)],
                        )
                nc.gpsimd.dma_start(out=wd_sb[:, 3], in_=wdt_d[idx, :, 3])
                sid = spool.tile([128, 2, BLOC], F32, tag="sid")
                tgd = spool.tile([128, 2, BLOC], F32, tag="tgd")
                sod = spool.tile([128, 2, BLOC], F32, tag="sod")
                for m in range(6):
                    ps = psum.tile([128, BLOC], F32, tag="ps")
                    for k in range(4):
                        nc.tensor.matmul(
                            ps,
                            lhsT=wd_sb[:, k, ts(m, 128)],
                            rhs=enc_bf[:, tsrc, k],
                            start=(k == 0),
                            stop=(k == 3),
                        )
                    dst = (
                        sid[:, m]
                        if m < 2
                        else tgd[:, m - 2]
                        if m < 4
                        else sod[:, m - 4]
                    )
                    nc.vector.tensor_scalar_add(dst, ps, bdec_sb[:, idx, m : m + 1])
                nc.scalar.activation(out=sid, in_=sid, func=AF.Sigmoid)
                nc.scalar.activation(out=tgd, in_=tgd, func=AF.Tanh)
                nc.scalar.activation(out=sod, in_=sod, func=AF.Sigmoid)
                nc.vector.tensor_mul(sid, sid, tgd)  # c
                nc.scalar.activation(out=sid, in_=sid, func=AF.Tanh)
                if idx % RES == 0:
                    run = spool.tile([128, 2, BLOC], F32, tag="run")
                    nc.vector.tensor_mul(run, sid, sod)
                else:
                    nc.vector.tensor_mul(sid, sid, sod)  # hd
                    nc.vector.tensor_add(run, run, sid)
                outv = spool.tile([128, 2, BLOC], F32, tag="outv")
                nc.scalar.activation(out=outv, in_=run, func=AF.Tanh)
                nc.sync.dma_start(out=out_d[idx], in_=outv)
    nc.finalize()
    return nc


def _host_prep(inputs):
    bf = ml_dtypes.bfloat16
    W_all = np.concatenate([inputs["Wih_enc"], inputs["Whh_enc"]], axis=2)
    wt = np.ascontiguousarray(
        W_all.transpose(0, 2, 1).reshape(S, 6, 128, 16 * 128).transpose(0, 2, 1, 3)
    ).astype(bf)
    benc = np.ascontiguousarray(
        (inputs["bih_enc"] + inputs["bhh_enc"]).reshape(S, 16, 128).transpose(2, 0, 1)
    ).astype(np.float32)
    Wd = inputs["Wih_dec"]
    Wd2 = np.concatenate([Wd[:, 0:256], Wd[:, 512:1024]], axis=1)
    wdt = np.ascontiguousarray(
        Wd2.transpose(0, 2, 1).reshape(S, 4, 128, 6 * 128).transpose(0, 2, 1, 3)
    ).astype(bf)
    bd = inputs["bih_dec"] + inputs["bhh_dec"]
    bdec = np.ascontiguousarray(
        np.concatenate([bd[:, 0:256], bd[:, 512:1024]], axis=1)
        .reshape(S, 6, 128)
        .transpose(2, 0, 1)
    ).astype(np.float32)
    xr = np.ascontiguousarray(
        inputs["x"].reshape(B, 2, 128, S).transpose(2, 3, 1, 0)
    ).astype(bf)
    return wt, benc, wdt, bdec, xr


def kernel(**inputs):
    inputs = {k: np.asarray(v) for k, v in inputs.items()}
    if "nc" not in _STATE:
        _STATE["nc"] = _build_module()
    nc = _STATE["nc"]
    wt, benc, wdt, bdec, xr = _host_prep(inputs)
    in_maps = []
    for c in range(NCORES):
        in_maps.append(
            {
                "wt": wt,
                "wdt": wdt,
                "benc": benc,
                "bdec": bdec,
                "xr": np.ascontiguousarray(xr[:, :, :, c * BLOC : (c + 1) * BLOC]),
            }
        )
    res = bass_utils.run_bass_kernel_spmd(
        nc,
        in_maps,
        core_ids=list(range(NCORES)),
        trace=bool(int(os.environ.get("BASS_KERNEL_TRACE", "0"))),
    )
    _STATE["last_results"] = res
    outs = []
    for c in range(NCORES):
        o = res.results[c]["out"]  # [S,128,2,BLOC]
        outs.append(
            np.ascontiguousarray(
                o.transpose(3, 2, 1, 0).reshape(BLOC, 2 * 128, S)[:, :, ::-1]
            )
        )
    return np.concatenate(outs, axis=0).astype(np.float32)
